# revision 1
# baseline (speedup 1.0000x reference)
"""Trainium2 Bass kernel for hierarchical loss.

Math: reference computes
    probs = outputs @ A.T            [B, N]
    w     = W[target]                [B, N]
    loss  = sum_b (1 - probs[b].w[b])
Since probs[b].w[b] = outputs[b] @ (A.T @ W[target_b]) = outputs[b] . M[target_b]
with M = W @ A  ([1000, 1000]), the loss is
    loss = B - sum_b outputs[b] . M[target_b]

M's entries are sums of a few powers of two (W entries are dyadic
rationals, A entries are 0/1), so M is exactly representable in bf16.

Device kernel (per core, data-parallel over batch):
  - stream 128-row tiles of outputs (bf16, padded to 1024 classes)
  - gather the 128 matching M rows by target via indirect DMA
  - for each 128-class chunk k: psum[c1,c2] += sum_b O[b,c1] * G[b,c2]
    (TensorE matmul, PSUM accumulation across all tiles and chunks)
  - the diagonal of the accumulated psum holds sum_b win[b] contributions;
    reduce it to a [128,1] vector, host sums across cores.
"""

import numpy as np
import ml_dtypes

NCORES = 8
B = 32768
C = 1000          # real classes
CP = 1024         # padded classes
P = 128
BPC = B // NCORES  # rows per core (4096)
NTILES = BPC // P  # 32

_NC_CACHE = {}


def _build(repeats=1, gather_mode="indirect"):
    import concourse.bass as bass
    import concourse.tile as tile
    from concourse import bacc, mybir
    from concourse.masks import make_identity

    nc = bacc.Bacc("TRN2", target_bir_lowering=False, debug=False,
                   num_devices=NCORES)
    o_ap = nc.dram_tensor("o", [BPC, CP], mybir.dt.bfloat16,
                          kind="ExternalInput").ap()
    m_ap = nc.dram_tensor("m", [C, CP], mybir.dt.bfloat16,
                          kind="ExternalInput").ap()
    t_ap = nc.dram_tensor("t", [P, NTILES], mybir.dt.int32,
                          kind="ExternalInput").ap()
    r_ap = nc.dram_tensor("r", [P, 1], mybir.dt.float32,
                          kind="ExternalOutput").ap()

    nchunk = CP // P

    with tile.TileContext(nc) as tc:
        with tc.tile_pool(name="io", bufs=4) as io_pool, \
             tc.tile_pool(name="single", bufs=1) as single, \
             tc.tile_pool(name="psum", bufs=1, space="PSUM") as psum_pool:
            t_sb = single.tile([P, NTILES], mybir.dt.int32)
            nc.sync.dma_start(t_sb[:], t_ap[:])

            acc = psum_pool.tile([P, P], mybir.dt.float32)
            n_mm = 0
            total_mm = repeats * NTILES * nchunk
            for _rep in range(repeats):
                for i in range(NTILES):
                    o_t = io_pool.tile([P, CP], mybir.dt.bfloat16, tag="o")
                    nc.sync.dma_start(o_t[:], o_ap[i * P:(i + 1) * P, :])
                    g_t = io_pool.tile([P, CP], mybir.dt.bfloat16, tag="g")
                    nc.gpsimd.indirect_dma_start(
                        out=g_t[:], out_offset=None, in_=m_ap[:],
                        in_offset=bass.IndirectOffsetOnAxis(
                            ap=t_sb[:, i:i + 1], axis=0))
                    for k in range(nchunk):
                        nc.tensor.matmul(
                            acc[:],
                            o_t[:, k * P:(k + 1) * P],
                            g_t[:, k * P:(k + 1) * P],
                            start=(n_mm == 0),
                            stop=(n_mm == total_mm - 1))
                        n_mm += 1

            ident = single.tile([P, P], mybir.dt.float32)
            make_identity(nc, ident[:])
            d_t = single.tile([P, P], mybir.dt.float32)
            r_t = single.tile([P, 1], mybir.dt.float32)
            nc.vector.tensor_tensor(
                out=d_t[:], in0=acc[:], in1=ident[:],
                op=mybir.AluOpType.mult)
            if repeats != 1:
                nc.vector.tensor_scalar_mul(d_t[:], d_t[:], 1.0 / repeats)
            nc.vector.tensor_reduce(
                out=r_t[:], in_=d_t[:], axis=mybir.AxisListType.X,
                op=mybir.AluOpType.add)
            nc.sync.dma_start(r_ap[:], r_t[:])

    nc.compile()
    return nc


def _get_nc(repeats=1, gather_mode="indirect"):
    key = (repeats, gather_mode)
    if key not in _NC_CACHE:
        _NC_CACHE[key] = _build(repeats, gather_mode)
    return _NC_CACHE[key]


def _make_in_maps(outputs, target, M):
    bf16 = ml_dtypes.bfloat16
    O = np.zeros((B, CP), dtype=bf16)
    O[:, :C] = outputs.astype(bf16)
    Mp = np.zeros((C, CP), dtype=bf16)
    Mp[:, :C] = M.astype(bf16)
    # per-core targets laid out [P, NTILES]: t_core[p, i] = target[core*BPC + i*P + p]
    t32 = target.astype(np.int32).reshape(NCORES, NTILES, P).transpose(0, 2, 1)
    t32 = np.ascontiguousarray(t32)
    return [{"o": O[c * BPC:(c + 1) * BPC], "m": Mp, "t": t32[c]}
            for c in range(NCORES)]


def kernel(outputs, target, A, W):
    outputs = np.asarray(outputs, dtype=np.float32)
    target = np.asarray(target)
    A = np.asarray(A, dtype=np.float32)
    W = np.asarray(W, dtype=np.float32)
    assert outputs.shape == (B, C) and target.shape == (B,)

    M = W @ A  # [1000, 1000], exact in f32 (small dyadic rationals)

    from concourse.bass_utils import run_bass_kernel_spmd
    nc = _get_nc()
    in_maps = _make_in_maps(outputs, target, M)
    res = run_bass_kernel_spmd(nc, in_maps, list(range(NCORES)))
    total = sum(float(res.results[c]["r"].sum(dtype=np.float64))
                for c in range(NCORES))
    return np.float32(np.float64(B) - total)



# revision 18
# speedup vs baseline: 1.7598x; 1.7598x over previous
"""Trainium2 Bass kernel for hierarchical loss.

Math: reference computes
    probs = outputs @ A.T            [B, N]
    w     = W[target]                [B, N]
    loss  = sum_b (1 - probs[b].w[b])
Since probs[b].w[b] = outputs[b] @ (A.T @ W[target_b]) = outputs[b] . M[target_b]
with M = W @ A  ([1000, 1000]), the loss is
    loss = B - sum_b outputs[b] . M[target_b]
         = B - sum_t s_t . M[t],   s_t = sum_{b: target_b = t} outputs[b]

Device kernel (per core, target-range sharded):
  - host sorts batch rows by target; core c owns targets [125c, 125(c+1)),
    rows padded with zeros to NT tiles of 256 rows
  - rows stream as fp8 (e4m3) tiles [128, 2, 1000]; M's entries are dyadic
    sums in {0.5, 0.75, 0.875, 1.0}, exactly representable in fp8
  - GpSimd builds the one-hot stationary E on device (slot == iota compare);
    padding rows carry slot -1 so their E row is all zero
  - TensorE DoubleRow matmuls S += E^T O accumulate per-target sums S
    [125, 1000] fp32 in PSUM (two 500-col halves, one PSUM bank each)
  - VectorE multiplies S by the core's M rows; ScalarE Copy+accum reduces to
    r[125, 2]; host sums and returns B - total.
"""

import numpy as np
import ml_dtypes

NCORES = 8
B = 32768
C = 1000          # classes
P = 128           # partitions
TPC = 125         # targets per core (125 * 8 = 1000)
KT = 2            # DoubleRow k-subtiles per partition
RPT = P * KT      # rows per tile (256)
NT_DEFAULT = 17   # tiles per core -> 4352 row capacity
NH = 500          # matmul half width (<= 512 fp32 = one PSUM bank)

FP8 = ml_dtypes.float8_e4m3

_NC_CACHE = {}


def _build(repeats=1, ntiles=NT_DEFAULT):
    import concourse.tile as tile
    from concourse import bacc, mybir

    nc = bacc.Bacc("TRN2", target_bir_lowering=False, debug=False,
                   num_devices=NCORES)
    o_ap = nc.dram_tensor("o", [ntiles - 1, P, KT, C], mybir.dt.float8e4,
                          kind="ExternalInput").ap()
    # last tile holds only 128 rows; ships as one DMA with both column
    # halves contiguous per partition (B half first)
    ol_ap = nc.dram_tensor("ol", [P, 2, NH], mybir.dt.float8e4,
                           kind="ExternalInput").ap()
    # tl (per-row target slot, -1 pad) and q (iota row) packed in one tensor
    tq_ap = nc.dram_tensor("tq", [P, ntiles * KT + P], mybir.dt.int32,
                           kind="ExternalInput").ap()
    m_ap = nc.dram_tensor("m", [TPC, C], mybir.dt.float8e4,
                          kind="ExternalInput").ap()
    r_ap = nc.dram_tensor("r", [TPC, 2], mybir.dt.float32,
                          kind="ExternalOutput").ap()

    # o DMAs move 2 tiles at a time: fewer instructions -> less per-DMA
    # SEQ/HWDGE overhead, same total bytes. Tiles 0..ntiles-2 stream in
    # chunks; the last tile goes separately as two column halves.
    nfull = ntiles - 1
    chunks = [(i, min(2, nfull - i)) for i in range(0, nfull, 2)]

    with tile.TileContext(nc) as tc:
        with tc.tile_pool(name="io", bufs=4) as io_pool, \
             tc.tile_pool(name="single", bufs=1) as single, \
             tc.tile_pool(name="psum", bufs=1, space="PSUM") as psum_pool:
            tq_sb = single.tile([P, ntiles * KT + P], mybir.dt.int32)
            nc.gpsimd.dma_start(tq_sb[:], tq_ap[:])
            mg_sb = single.tile([TPC, C], mybir.dt.float8e4)
            nc.gpsimd.dma_start(mg_sb[:], m_ap[:])
            q_bc = tq_sb[:, ntiles * KT:][:, None, :].broadcast_to([P, KT, P])

            # one-hot stationaries, built once on VectorE
            e_sb = single.tile([P, ntiles, KT, P], mybir.dt.float8e4)
            for i in range(ntiles):
                nc.vector.tensor_tensor(
                    out=e_sb[:, i, :, :],
                    in0=tq_sb[:, KT * i:KT * (i + 1)][:, :, None]
                        .broadcast_to([P, KT, P]),
                    in1=q_bc,
                    op=mybir.AluOpType.is_equal)

            acc_a = psum_pool.tile([P, 512], mybir.dt.float32)
            acc_b = psum_pool.tile([P, 512], mybir.dt.float32)
            sink = psum_pool.tile([P, 512], mybir.dt.float32)

            n_mm = 0
            mm_per_rep = 2 * ntiles
            last_a = repeats * mm_per_rep - 1   # final A matmul
            last_b = repeats * mm_per_rep - 2   # final B matmul (shipped first)
            lhs_l = e_sb[:, ntiles - 1, :, :]
            for _rep in range(repeats):
                for i0, csz in chunks:
                    o_t = io_pool.tile([P, csz, KT, C], mybir.dt.float8e4,
                                       tag="o")
                    nc.sync.dma_start(
                        o_t[:], o_ap[i0:i0 + csz].transpose([1, 0, 2, 3]))
                    for u in range(csz):
                        lhs = e_sb[:, i0 + u, :, :]
                        nc.tensor.matmul(
                            acc_a[:, :NH], lhs, o_t[:, u, :, 0:NH],
                            start=(n_mm == 0), stop=(n_mm == last_a),
                            perf_mode=mybir.MatmulPerfMode.DoubleRow)
                        n_mm += 1
                        nc.tensor.matmul(
                            acc_b[:, :NH], lhs, o_t[:, u, :, NH:C],
                            start=(n_mm == 1), stop=(n_mm == last_b),
                            perf_mode=mybir.MatmulPerfMode.DoubleRow)
                        n_mm += 1
                # last tile: B half first so acc_b closes early
                ob_t = io_pool.tile([P, KT, NH], mybir.dt.float8e4, tag="ob")
                nc.sync.dma_start(ob_t[:], ol_ap[0])
                nc.tensor.matmul(
                    acc_b[:, :NH], lhs_l, ob_t[:],
                    start=False, stop=(n_mm == last_b),
                    perf_mode=mybir.MatmulPerfMode.DoubleRow)
                n_mm += 1
                oa_t = io_pool.tile([P, KT, NH], mybir.dt.float8e4, tag="oa")
                nc.sync.dma_start(oa_t[:], ol_ap[1])
                nc.tensor.matmul(
                    acc_a[:, :NH], lhs_l, oa_t[:],
                    start=False, stop=(n_mm == last_a),
                    perf_mode=mybir.MatmulPerfMode.DoubleRow)
                n_mm += 1

            prod_a = single.tile([TPC, NH], mybir.dt.float32)
            prod_b = single.tile([TPC, NH], mybir.dt.float32)
            r_t = single.tile([TPC, 2], mybir.dt.float32)
            scale = 1.0 / repeats
            # B closes first: DVE mult; A next: DVE mult + ScalarE accumulate
            # runs while DVE reduces B
            nc.vector.tensor_tensor(
                out=prod_b[:], in0=acc_b[:TPC, :NH], in1=mg_sb[:, NH:C],
                op=mybir.AluOpType.mult)
            nc.vector.tensor_tensor(
                out=prod_a[:], in0=acc_a[:TPC, :NH], in1=mg_sb[:, 0:NH],
                op=mybir.AluOpType.mult)
            nc.scalar.activation(
                out=sink[:TPC, :NH], in_=prod_a[:],
                func=mybir.ActivationFunctionType.Copy, scale=scale,
                accum_out=r_t[:, 0:1])
            nc.vector.tensor_reduce(
                out=r_t[:, 1:2], in_=prod_b[:], axis=mybir.AxisListType.X,
                op=mybir.AluOpType.add)
            if repeats != 1:
                nc.vector.tensor_scalar_mul(
                    r_t[:, 1:2], r_t[:, 1:2], scale)
            nc.scalar.dma_start(r_ap[:], r_t[:])

    nc.compile()
    return nc


def _get_nc(repeats=1, ntiles=NT_DEFAULT):
    key = (repeats, ntiles)
    if key not in _NC_CACHE:
        _NC_CACHE[key] = _build(repeats, ntiles)
    return _NC_CACHE[key]


def _shard(outputs, target, M):
    """Sort rows by target, shard by target range, pack fp8 tiles.

    Returns (in_maps, ntiles)."""
    t = np.asarray(target).astype(np.int64)
    order = np.argsort(t, kind="stable")
    t_sorted = t[order]
    bounds = np.searchsorted(t_sorted, np.arange(0, 1001, TPC))
    max_rows = int(np.max(bounds[1:] - bounds[:-1]))
    ntiles = max(NT_DEFAULT, -(-max_rows // RPT))

    o8 = outputs.astype(FP8)
    m8 = M.astype(FP8)
    q = np.broadcast_to(np.arange(P, dtype=np.int32), (P, P))
    in_maps = []
    for c in range(NCORES):
        lo, hi = int(bounds[c]), int(bounds[c + 1])
        rows = order[lo:hi]
        n = hi - lo
        O_core = np.zeros((ntiles * RPT, C), dtype=FP8)
        O_core[:n] = o8[rows]
        O_core = O_core.reshape(ntiles, KT, P, C).transpose(0, 2, 1, 3)
        O_main = np.ascontiguousarray(O_core[:ntiles - 1])
        O_last = O_core[ntiles - 1]            # [P, KT, C]
        O_l = np.ascontiguousarray(
            np.stack([O_last[:, :, NH:C], O_last[:, :, 0:NH]]))
        slot = np.full(ntiles * RPT, -1, dtype=np.int32)
        slot[:n] = (t_sorted[lo:hi] - TPC * c).astype(np.int32)
        tl = slot.reshape(ntiles, KT, P).transpose(2, 0, 1).reshape(P, -1)
        tq = np.ascontiguousarray(np.concatenate([tl, q], axis=1))
        in_maps.append({"o": O_main, "ol": O_l, "tq": tq,
                        "m": m8[TPC * c:TPC * (c + 1)]})
    return in_maps, ntiles


def kernel(outputs, target, A, W):
    outputs = np.asarray(outputs, dtype=np.float32)
    target = np.asarray(target)
    A = np.asarray(A, dtype=np.float32)
    W = np.asarray(W, dtype=np.float32)
    assert outputs.shape == (B, C) and target.shape == (B,)

    M = W @ A  # [1000, 1000]

    from concourse.bass_utils import run_bass_kernel_spmd
    in_maps, ntiles = _shard(outputs, target, M)
    nc = _get_nc(1, ntiles)
    res = run_bass_kernel_spmd(nc, in_maps, list(range(NCORES)))
    total = sum(float(res.results[c]["r"].sum(dtype=np.float64))
                for c in range(NCORES))
    return np.float32(np.float64(B) - total)


# revision 25
# speedup vs baseline: 5.6407x; 3.2052x over previous
"""Trainium2 Bass kernel for hierarchical loss.

Math: reference computes
    probs = outputs @ A.T            [B, N]
    w     = W[target]                [B, N]
    loss  = sum_b (1 - probs[b].w[b])
Since probs[b].w[b] = outputs[b] @ (A.T @ W[target_b]) = outputs[b] . M[target_b]
with M = W @ A  ([1000, 1000]), the loss is
    loss = B - sum_b outputs[b] . M[target_b]
         = B - sum_t s_t . M[t],   s_t = sum_{b: target_b = t} outputs[b]

Device kernel (per core, target-range sharded):
  - host sorts batch rows by target; core c owns targets [125c, 125(c+1)),
    rows padded with zeros to NT tiles of 256 rows
  - rows stream as fp8 (e4m3) tiles [128, 2, 1000]; M's entries are dyadic
    sums in {0.5, 0.75, 0.875, 1.0}, exactly representable in fp8
  - VectorE builds the one-hot stationary E on device (slot == iota compare);
    padding rows carry slot -1 so their E row is all zero
  - TensorE DoubleRow matmuls S += E^T O accumulate per-target sums S
    [125, 1000] fp32 in PSUM (two 500-col halves, one PSUM bank each)
  - VectorE multiplies S by the core's M rows; ScalarE Copy+accum reduces to
    r[125, 2]; host sums and returns B - total.
"""

import numpy as np
import ml_dtypes

NCORES = 8
B = 32768
C = 1000          # classes
P = 128           # partitions
TPC = 125         # targets per core (125 * 8 = 1000)
KT = 2            # DoubleRow k-subtiles per partition
RPT = P * KT      # rows per tile (256)
NT_DEFAULT = 17   # tiles per core -> 4352 row capacity
NH = 500          # matmul half width (<= 512 fp32 = one PSUM bank)

FP8 = ml_dtypes.float8_e4m3

_NC_CACHE = {}


def _build(repeats=1, ntiles=NT_DEFAULT):
    import concourse.tile as tile
    from concourse import bacc, mybir

    nc = bacc.Bacc("TRN2", target_bir_lowering=False, debug=False,
                   num_devices=NCORES)
    o_ap = nc.dram_tensor("o", [ntiles - 1, P, KT, C], mybir.dt.float8e4,
                          kind="ExternalInput").ap()
    # last tile ships as two contiguous column halves (B half first) so each
    # accumulation group can close as early as possible
    ol_ap = nc.dram_tensor("ol", [2, P, KT, NH], mybir.dt.float8e4,
                           kind="ExternalInput").ap()
    # tl (per-row target slot, -1 pad) and q (iota row) packed in one tensor
    tq_ap = nc.dram_tensor("tq", [P, ntiles * KT + P], mybir.dt.int32,
                           kind="ExternalInput").ap()
    m_ap = nc.dram_tensor("m", [TPC, C], mybir.dt.float8e4,
                          kind="ExternalInput").ap()
    r_ap = nc.dram_tensor("r", [TPC, 2], mybir.dt.float32,
                          kind="ExternalOutput").ap()

    # o DMAs move 2 tiles at a time: fewer instructions -> less per-DMA
    # SEQ/HWDGE overhead, same total bytes. Tiles 0..ntiles-2 stream in
    # chunks; the last tile goes separately as two column halves.
    nfull = ntiles - 1
    chunks = [(i, min(2, nfull - i)) for i in range(0, nfull, 2)]

    with tile.TileContext(nc) as tc:
        with tc.tile_pool(name="io", bufs=4) as io_pool, \
             tc.tile_pool(name="single", bufs=1) as single, \
             tc.tile_pool(name="psum", bufs=1, space="PSUM") as psum_pool:
            tq_sb = single.tile([P, ntiles * KT + P], mybir.dt.int32)
            nc.gpsimd.dma_start(tq_sb[:], tq_ap[:])
            mg_sb = single.tile([TPC, C], mybir.dt.float8e4)
            nc.gpsimd.dma_start(mg_sb[:], m_ap[:])
            q_bc = tq_sb[:, ntiles * KT:][:, None, :].broadcast_to([P, KT, P])

            # one-hot stationaries, built once on VectorE
            e_sb = single.tile([P, ntiles, KT, P], mybir.dt.float8e4)
            for i in range(ntiles):
                nc.vector.tensor_tensor(
                    out=e_sb[:, i, :, :],
                    in0=tq_sb[:, KT * i:KT * (i + 1)][:, :, None]
                        .broadcast_to([P, KT, P]),
                    in1=q_bc,
                    op=mybir.AluOpType.is_equal)

            acc_a = psum_pool.tile([P, 512], mybir.dt.float32)
            acc_b = psum_pool.tile([P, 512], mybir.dt.float32)
            sink = psum_pool.tile([P, 512], mybir.dt.float32)

            n_mm = 0
            mm_per_rep = 2 * ntiles
            last_a = repeats * mm_per_rep - 1   # final A matmul
            last_b = repeats * mm_per_rep - 2   # final B matmul (shipped first)
            lhs_l = e_sb[:, ntiles - 1, :, :]
            for _rep in range(repeats):
                for i0, csz in chunks:
                    o_t = io_pool.tile([P, csz, KT, C], mybir.dt.float8e4,
                                       tag="o")
                    nc.sync.dma_start(
                        o_t[:], o_ap[i0:i0 + csz].transpose([1, 0, 2, 3]))
                    for u in range(csz):
                        lhs = e_sb[:, i0 + u, :, :]
                        nc.tensor.matmul(
                            acc_a[:, :NH], lhs, o_t[:, u, :, 0:NH],
                            start=(n_mm == 0), stop=(n_mm == last_a),
                            perf_mode=mybir.MatmulPerfMode.DoubleRow)
                        n_mm += 1
                        nc.tensor.matmul(
                            acc_b[:, :NH], lhs, o_t[:, u, :, NH:C],
                            start=(n_mm == 1), stop=(n_mm == last_b),
                            perf_mode=mybir.MatmulPerfMode.DoubleRow)
                        n_mm += 1
                # last tile: B half first so acc_b closes early
                ob_t = io_pool.tile([P, KT, NH], mybir.dt.float8e4, tag="ob")
                nc.sync.dma_start(ob_t[:], ol_ap[0])
                nc.tensor.matmul(
                    acc_b[:, :NH], lhs_l, ob_t[:],
                    start=False, stop=(n_mm == last_b),
                    perf_mode=mybir.MatmulPerfMode.DoubleRow)
                n_mm += 1
                oa_t = io_pool.tile([P, KT, NH], mybir.dt.float8e4, tag="oa")
                nc.sync.dma_start(oa_t[:], ol_ap[1])
                nc.tensor.matmul(
                    acc_a[:, :NH], lhs_l, oa_t[:],
                    start=False, stop=(n_mm == last_a),
                    perf_mode=mybir.MatmulPerfMode.DoubleRow)
                n_mm += 1

            prod_a = single.tile([TPC, NH], mybir.dt.float32)
            prod_b = single.tile([TPC, NH], mybir.dt.float32)
            r_t = single.tile([TPC, 2], mybir.dt.float32)
            scale = 1.0 / repeats
            # B closes first: DVE mult; A next: DVE mult + ScalarE accumulate
            # runs while DVE reduces B
            nc.vector.tensor_tensor(
                out=prod_b[:], in0=acc_b[:TPC, :NH], in1=mg_sb[:, NH:C],
                op=mybir.AluOpType.mult)
            nc.vector.tensor_tensor(
                out=prod_a[:], in0=acc_a[:TPC, :NH], in1=mg_sb[:, 0:NH],
                op=mybir.AluOpType.mult)
            nc.scalar.activation(
                out=sink[:TPC, :NH], in_=prod_a[:],
                func=mybir.ActivationFunctionType.Copy, scale=scale,
                accum_out=r_t[:, 0:1])
            nc.vector.tensor_reduce(
                out=r_t[:, 1:2], in_=prod_b[:], axis=mybir.AxisListType.X,
                op=mybir.AluOpType.add)
            if repeats != 1:
                nc.vector.tensor_scalar_mul(
                    r_t[:, 1:2], r_t[:, 1:2], scale)
            nc.scalar.dma_start(r_ap[:], r_t[:])

    nc.compile()
    return nc


def _get_nc(repeats=1, ntiles=NT_DEFAULT):
    key = (repeats, ntiles)
    if key not in _NC_CACHE:
        _NC_CACHE[key] = _build(repeats, ntiles)
    return _NC_CACHE[key]


def _shard(outputs, target, M):
    """Sort rows by target, shard by target range, pack fp8 tiles.

    Returns (in_maps, ntiles)."""
    t = np.asarray(target).astype(np.int64)
    order = np.argsort(t, kind="stable")
    t_sorted = t[order]
    bounds = np.searchsorted(t_sorted, np.arange(0, 1001, TPC))
    max_rows = int(np.max(bounds[1:] - bounds[:-1]))
    ntiles = max(NT_DEFAULT, -(-max_rows // RPT))
    assert ntiles * RPT >= max_rows

    o8 = outputs.astype(FP8)
    m8 = M.astype(FP8)
    q = np.broadcast_to(np.arange(P, dtype=np.int32), (P, P))
    in_maps = []
    for c in range(NCORES):
        lo, hi = int(bounds[c]), int(bounds[c + 1])
        rows = order[lo:hi]
        n = hi - lo
        O_core = np.zeros((ntiles * RPT, C), dtype=FP8)
        O_core[:n] = o8[rows]
        O_core = O_core.reshape(ntiles, KT, P, C).transpose(0, 2, 1, 3)
        O_main = np.ascontiguousarray(O_core[:ntiles - 1])
        O_last = O_core[ntiles - 1]            # [P, KT, C]
        O_l = np.ascontiguousarray(
            np.stack([O_last[:, :, NH:C], O_last[:, :, 0:NH]]))
        slot = np.full(ntiles * RPT, -1, dtype=np.int32)
        slot[:n] = (t_sorted[lo:hi] - TPC * c).astype(np.int32)
        tl = slot.reshape(ntiles, KT, P).transpose(2, 0, 1).reshape(P, -1)
        tq = np.ascontiguousarray(np.concatenate([tl, q], axis=1))
        in_maps.append({"o": O_main, "ol": O_l, "tq": tq,
                        "m": m8[TPC * c:TPC * (c + 1)]})
    return in_maps, ntiles


def kernel(outputs, target, A, W):
    outputs = np.asarray(outputs, dtype=np.float32)
    target = np.asarray(target)
    A = np.asarray(A, dtype=np.float32)
    W = np.asarray(W, dtype=np.float32)
    assert outputs.shape == (B, C) and target.shape == (B,)

    M = W @ A  # [1000, 1000]

    from concourse.bass_utils import run_bass_kernel_spmd
    in_maps, ntiles = _shard(outputs, target, M)
    nc = _get_nc(1, ntiles)
    res = run_bass_kernel_spmd(nc, in_maps, list(range(NCORES)))
    total = sum(float(res.results[c]["r"].sum(dtype=np.float64))
                for c in range(NCORES))
    return np.float32(np.float64(B) - total)
